# revision 1
# baseline (speedup 1.0000x reference)
"""Trainium2 Bass kernel for nn_MeshDeformation (GNN message passing).

Strategy (data-parallel over batch B=8 across 8 cores, one batch item/core):
  - Activations vertex-major bf16 in SBUF; per-conv PE transpose builds the
    feat-major copy used as matmul lhsT.
  - gconv: mm = x@W (PE) -> mm to HBM (bf16 rows) -> dma_gather pulls the
    dst-sorted, per-dst-block-padded edge rows edge-major into SBUF ->
    scatter matmul per 128-edge k-tile with a static S matrix (val folded
    in, streamed from HBM) accumulating in PSUM per dst block, plus the
    x@L term and bias in the same PSUM group -> fused ReLU evacuation.
  - conv2 uses spmm(x)@W2 == spmm(x@W2) commutation so the gather stays on
    256-wide rows; tanh*0.1 fused into the final evacuation.
"""
import sys, os
sys.path.insert(0, '/opt/trn_rl_repo')
import numpy as np
import ml_dtypes

import concourse.bass as bass
import concourse.bacc as bacc
import concourse.mybir as mybir
import concourse.tile as tile
from concourse import bass_utils

N = 6890
NP = 6912          # padded vertices (54 * 128)
NB = NP // 128     # 54 dst/vertex blocks
E = 41340
HID = 256
FEAT = 128
NCONV = 10         # conv1, 8 hidden convs, final conv2
DEBUG_STAGE = 0
CH = 32            # gather/scatter k-tiles per chunk

BF16 = ml_dtypes.bfloat16


def _edge_tiles(src, dst, val):
    """dst-sorted, per-dst-block 128-padded edge tiling.

    Returns (gidx_flat [KT*128] int16 src ids, S [KT,128,128] bf16,
    tile_block [KT] int).
    """
    order = np.argsort(dst, kind='stable')
    src, dst, val = src[order], dst[order], val[order]
    gidx, s_tiles, tile_block = [], [], []
    for b in range(NB):
        lo = np.searchsorted(dst, b * 128)
        hi = np.searchsorted(dst, (b + 1) * 128)
        eb_src = src[lo:hi]
        eb_dst = dst[lo:hi] - b * 128
        eb_val = val[lo:hi]
        cnt = hi - lo
        if cnt == 0:
            continue
        ntile = (cnt + 127) // 128
        pad = ntile * 128 - cnt
        eb_src = np.concatenate([eb_src, np.zeros(pad, np.int64)])
        for t in range(ntile):
            sl = slice(t * 128, (t + 1) * 128)
            gidx.append(eb_src[sl])
            S = np.zeros((128, 128), np.float32)
            for e in range(t * 128, min((t + 1) * 128, cnt)):
                S[e - t * 128, eb_dst[e]] += eb_val[e]
            s_tiles.append(S)
            tile_block.append(b)
    gidx = np.concatenate(gidx).astype(np.int16)
    S = np.stack(s_tiles).astype(BF16)
    return gidx, S, tile_block


def _wrap_idx(gidx, tile_block):
    """Per-k-tile partition-aligned int32 offsets [128, KT]: column j holds
    tile j's 128 source row ids (offset for output partition p at row p)."""
    KT = len(tile_block)
    nch = (KT + CH - 1) // CH
    out = gidx.astype(np.int32).reshape(KT, 128).T.copy()
    return out, nch


def _build_program(tile_block, nch, chunk_tiles):
    KT = len(tile_block)
    nc = bacc.Bacc("TRN2", target_bir_lowering=False, debug=False)
    bf = mybir.dt.bfloat16
    f32 = mybir.dt.float32

    x0_d = nc.dram_tensor("x0", [NP, FEAT], f32, kind="ExternalInput")
    wcat_d = nc.dram_tensor("wcat", [128, NCONV * 2 * HID], bf, kind="ExternalInput")
    lcat_d = nc.dram_tensor("lcat", [128, NCONV * 2 * HID], bf, kind="ExternalInput")
    bias_d = nc.dram_tensor("bias", [(NCONV + 1) * HID], bf, kind="ExternalInput")
    s_d = nc.dram_tensor("smat", [KT, 128, 128], bf, kind="ExternalInput")
    gidx_d = nc.dram_tensor("gidx", [128, KT], mybir.dt.int32,
                            kind="ExternalInput")
    out_d = nc.dram_tensor("out", [N, 3], f32, kind="ExternalOutput")
    if DEBUG_STAGE >= 1:
        dbg_d = nc.dram_tensor("dbg", [128, NB * HID], bf, kind="ExternalOutput")
    if DEBUG_STAGE == 8:
        dbg2_d = nc.dram_tensor("dbg2", [NP, HID], bf, kind="ExternalOutput")

    from concourse.masks import make_identity

    with tile.TileContext(nc) as tc:
        with (
            tc.tile_pool(name="dram", bufs=1, space="DRAM") as dram,
            tc.tile_pool(name="res", bufs=1) as res,
            tc.tile_pool(name="sstage", bufs=2) as sstage,
            tc.tile_pool(name="gpool", bufs=8) as gpool,
            tc.tile_pool(name="stg", bufs=3) as stg,
            tc.tile_pool(name="acc", bufs=3, space="PSUM") as acc,
            tc.tile_pool(name="tp", bufs=2, space="PSUM") as tp,
            tc.tile_pool(name="pout", bufs=2, space="PSUM") as pout,
        ):
            mm_hbm = dram.tile([NP, HID], bf)

            xT = res.tile([128, 2 * NP], bf, tag="xT")
            A = res.tile([128, NB * HID], bf, tag="A")
            B = res.tile([128, NB * HID], bf, tag="B")
            wc = res.tile([128, NCONV * 2 * HID], bf, tag="wc")
            lc = res.tile([128, NCONV * 2 * HID], bf, tag="lc")
            brow = res.tile([1, (NCONV + 1) * HID], bf, tag="brow")
            ones1 = res.tile([1, 128], bf, tag="ones1")
            gidx_t = res.tile([128, KT], mybir.dt.int32, tag="gidx")
            id32 = res.tile([128, 128], f32, tag="id32")
            idbf = res.tile([128, 128], bf, tag="idbf")

            nc.sync.dma_start(out=wc[:], in_=wcat_d[:])
            nc.sync.dma_start(out=lc[:], in_=lcat_d[:])
            nc.sync.dma_start(out=brow[:], in_=bias_d[:][None, :])
            nc.sync.dma_start(out=gidx_t[:], in_=gidx_d[:])
            make_identity(nc, id32[:])
            nc.vector.tensor_copy(out=idbf[:], in_=id32[:])
            nc.gpsimd.memset(ones1[:], 1.0)

            def transpose_into_xT(read_block, fin_tiles):
                """read_block(i) -> AP [128, fin_tiles*128] vertex-major chunk."""
                for i in range(NB):
                    chunk = read_block(i)
                    for h in range(fin_tiles):
                        pt = tp.tile([128, 128], bf)
                        nc.tensor.transpose(
                            out=pt[:], in_=chunk[:, h * 128:(h + 1) * 128],
                            identity=idbf[:])
                        nc.vector.tensor_copy(
                            out=xT[:, h * NP + i * 128: h * NP + (i + 1) * 128],
                            in_=pt[:])

            def conv(c, src_tile, dst_mode):
                """One graph conv. src_tile: vertex-major bf16 [128, NB*HID]
                (None for conv0 -> x0 HBM f32). dst_mode: 'A','B','resid','final'.
                """
                fin_tiles = 1 if c == 0 else 2

                # --- phase T: build feat-major xT from the conv input ---
                if c == 0:
                    def rd(i):
                        t = stg.tile([128, FEAT], f32, tag="x0st")
                        nc.sync.dma_start(
                            out=t[:], in_=x0_d[i * 128:(i + 1) * 128, :])
                        return t

                    def rd_tr(i):
                        chunk = rd(i)
                        pt = tp.tile([128, 128], f32)
                        nc.tensor.transpose(out=pt[:], in_=chunk[:],
                                            identity=id32[:])
                        nc.vector.tensor_copy(
                            out=xT[:, i * 128:(i + 1) * 128], in_=pt[:])
                    for i in range(NB):
                        rd_tr(i)
                else:
                    transpose_into_xT(
                        lambda i: src_tile[:, i * HID:(i + 1) * HID], fin_tiles)

                # --- phase M: mm = x@W -> mm_hbm (bf16 rows) ---
                if dst_mode == 'final':
                    # conv2 commutation: gather x itself
                    nc.sync.dma_start(
                        out=mm_hbm[:].rearrange("(i p) f -> p i f", p=128),
                        in_=src_tile[:].rearrange("p (i f) -> p i f", f=HID))
                else:
                    for i in range(NB):
                        pm = acc.tile([128, HID], f32, tag="pacc")
                        for h in range(fin_tiles):
                            nc.tensor.matmul(
                                out=pm[:],
                                lhsT=xT[:, h * NP + i * 128: h * NP + (i + 1) * 128],
                                rhs=wc[:, (2 * c + h) * HID:(2 * c + h + 1) * HID],
                                start=(h == 0), stop=(h == fin_tiles - 1))
                        ms = stg.tile([128, HID], bf, tag="mmst")
                        nc.scalar.copy(out=ms[:], in_=pm[:])
                        nc.sync.dma_start(
                            out=mm_hbm[i * 128:(i + 1) * 128, :], in_=ms[:])

                # mm_hbm writes must land before gathers read (DRAM RAW)
                tc.strict_bb_all_engine_barrier()

                # --- phase G+S: gather chunks + scatter matmuls ---
                fout = HID
                cur_blk = -1
                pacc = None

                def finish_block(i, first):
                    # L-term + bias into the same psum group, then evacuate.
                    # 'final' keeps pacc = pure spmm (L2/bias applied in po);
                    # the ones x zero-slot matmul just closes the psum group.
                    if dst_mode != 'final':
                        for h in range(fin_tiles):
                            nc.tensor.matmul(
                                out=pacc[:],
                                lhsT=xT[:, h * NP + i * 128: h * NP + (i + 1) * 128],
                                rhs=lc[:, (2 * c + h) * HID:(2 * c + h + 1) * HID],
                                start=first and h == 0, stop=False)
                    bslot = NCONV if dst_mode == 'final' else c
                    nc.tensor.matmul(
                        out=pacc[:], lhsT=ones1[:],
                        rhs=brow[:, bslot * HID:(bslot + 1) * HID],
                        start=first and dst_mode == 'final', stop=True)
                    sl = slice(i * HID, (i + 1) * HID)
                    if dst_mode == 'A':
                        nc.scalar.activation(
                            out=A[:, sl], in_=pacc[:],
                            func=mybir.ActivationFunctionType.Relu)
                    elif dst_mode == 'B':
                        nc.scalar.activation(
                            out=B[:, sl], in_=pacc[:],
                            func=mybir.ActivationFunctionType.Relu)
                    elif dst_mode == 'resid':
                        t = stg.tile([128, HID], bf, tag="rst")
                        nc.scalar.activation(
                            out=t[:], in_=pacc[:],
                            func=mybir.ActivationFunctionType.Relu)
                        nc.vector.tensor_tensor(
                            out=A[:, sl], in0=A[:, sl], in1=t[:],
                            op=mybir.AluOpType.add)
                        nc.scalar.mul(out=A[:, sl], in_=A[:, sl], mul=0.5)
                    else:  # 'final': s2 block -> tiny matmuls -> tanh out
                        t = B[:, sl]
                        nc.scalar.copy(out=t, in_=pacc[:])
                        s2T = stg.tile([128, 256], bf, tag="s2T")
                        for h in range(2):
                            pt = tp.tile([128, 128], bf)
                            nc.tensor.transpose(
                                out=pt[:], in_=B[:, i * HID + h * 128:
                                                 i * HID + (h + 1) * 128],
                                identity=idbf[:])
                            nc.vector.tensor_copy(
                                out=s2T[:, h * 128:(h + 1) * 128], in_=pt[:])
                        po = pout.tile([128, 3], f32)
                        for h in range(2):
                            nc.tensor.matmul(
                                out=po[:], lhsT=s2T[:, h * 128:(h + 1) * 128],
                                rhs=wc[:, (2 * c + h) * HID:(2 * c + h) * HID + 3],
                                start=(h == 0), stop=False)
                            nc.tensor.matmul(
                                out=po[:],
                                lhsT=xT[:, h * NP + i * 128: h * NP + (i + 1) * 128],
                                rhs=lc[:, (2 * c + h) * HID:(2 * c + h) * HID + 3],
                                start=False, stop=False)
                        nc.tensor.matmul(
                            out=po[:], lhsT=ones1[:],
                            rhs=brow[:, c * HID: c * HID + 3],
                            start=False, stop=True)
                        ot = stg.tile([128, 3], f32, tag="outst")
                        nc.scalar.activation(
                            out=ot[:], in_=po[:],
                            func=mybir.ActivationFunctionType.Tanh)
                        nc.scalar.mul(out=ot[:], in_=ot[:], mul=0.1)
                        rows = min(128, N - i * 128)
                        nc.sync.dma_start(
                            out=out_d[i * 128: i * 128 + rows, :],
                            in_=ot[:rows, :])

                jglobal = 0
                for ci in range(nch):
                    nt = chunk_tiles[ci]
                    st = sstage.tile([128, CH * 128], bf, tag="S")
                    nc.sync.dma_start(
                        out=st[:].rearrange("p (j d) -> p j d", d=128)[:, :nt],
                        in_=s_d[jglobal:jglobal + nt].rearrange("j p d -> p j d"))
                    for jj in range(nt):
                        j = jglobal + jj
                        g = gpool.tile([128, fout], bf, tag="G")
                        nc.gpsimd.indirect_dma_start(
                            out=g[:], out_offset=None, in_=mm_hbm[:],
                            in_offset=bass.IndirectOffsetOnAxis(
                                ap=gidx_t[:, j:j + 1], axis=0))
                        blk = tile_block[j]
                        if blk != cur_blk:
                            if cur_blk >= 0:
                                finish_block(cur_blk, False)
                            cur_blk = blk
                            pacc = acc.tile([128, HID], f32, tag="pacc")
                            first_mm = True
                        nc.tensor.matmul(
                            out=pacc[:],
                            lhsT=st[:, jj * 128:(jj + 1) * 128],
                            rhs=g[:],
                            start=first_mm, stop=False)
                        first_mm = False
                    jglobal += nt
                if cur_blk >= 0:
                    finish_block(cur_blk, False)
                # blocks with zero edges never appear in tile_block: handle any
                # missing blocks with an L-only psum group
                seen = set(tile_block)
                for i in range(NB):
                    if i not in seen:
                        pacc = acc.tile([128, HID], f32, tag="pacc")
                        finish_block(i, True)
                # gathers must finish before the next conv rewrites mm_hbm
                tc.strict_bb_all_engine_barrier()

            conv(0, None, 'A')
            if DEBUG_STAGE == 1:
                nc.sync.dma_start(out=dbg_d[:], in_=A[:])
            elif DEBUG_STAGE == 2:
                conv(1, A, 'B')
                nc.sync.dma_start(out=dbg_d[:], in_=B[:])
            elif DEBUG_STAGE == 4:
                conv(9, A, 'final')
            elif DEBUG_STAGE == 3:
                conv(1, A, 'B')
                conv(2, B, 'resid')
                nc.sync.dma_start(out=dbg_d[:], in_=A[:])
            elif DEBUG_STAGE == 8:
                for b in range(4):
                    conv(2 * b + 1, A, 'B')
                    conv(2 * b + 2, B, 'resid')
                conv(9, A, 'final')
                nc.sync.dma_start(out=dbg_d[:], in_=B[:])
                nc.sync.dma_start(out=dbg2_d[:], in_=mm_hbm[:])
            elif DEBUG_STAGE in (5, 6, 7, 9):
                nblk = DEBUG_STAGE - 4 if DEBUG_STAGE < 9 else 4
                for b in range(nblk):
                    conv(2 * b + 1, A, 'B')
                    conv(2 * b + 2, B, 'resid')
                nc.sync.dma_start(out=dbg_d[:], in_=A[:])
            else:
                for b in range(4):
                    conv(2 * b + 1, A, 'B')
                    conv(2 * b + 2, B, 'resid')
                conv(9, A, 'final')

    nc.finalize()
    return nc


_CACHE = {}


def kernel(**inputs):
    verts = np.asarray(inputs["verts_feats"], np.float32)   # [8, 6890, 128]
    src = np.asarray(inputs["edge_src"]).astype(np.int64)
    dst = np.asarray(inputs["edge_dst"]).astype(np.int64)
    val = np.asarray(inputs["edge_val"], np.float32)
    Bsz = verts.shape[0]

    gidx, S, tile_block = _edge_tiles(src, dst, val)
    gidx_w, nch = _wrap_idx(gidx, tile_block)
    KT = len(tile_block)
    chunk_tiles = [min(CH, KT - c * CH) for c in range(nch)]

    # weight concatenation [128, 9*2*256] bf16
    wcat = np.zeros((128, NCONV * 2 * HID), np.float32)
    lcat = np.zeros((128, NCONV * 2 * HID), np.float32)
    bias = np.zeros((NCONV + 1) * HID, np.float32)

    def put(c, W, L, b, ncols=HID):
        for h in range(W.shape[0] // 128):
            wcat[:, (2 * c + h) * HID:(2 * c + h) * HID + ncols] = \
                W[h * 128:(h + 1) * 128, :ncols]
            lcat[:, (2 * c + h) * HID:(2 * c + h) * HID + ncols] = \
                L[h * 128:(h + 1) * 128, :ncols]
        bias[c * HID:c * HID + len(b)] = b

    put(0, np.asarray(inputs["W1"], np.float32), np.asarray(inputs["L1"], np.float32),
        np.asarray(inputs["b1"], np.float32))
    Wb = np.asarray(inputs["Wb"], np.float32)
    Lb = np.asarray(inputs["Lb"], np.float32)
    bb = np.asarray(inputs["bb"], np.float32)
    for k in range(8):
        put(1 + k, Wb[k], Lb[k], bb[k])
    put(9, np.asarray(inputs["W2"], np.float32), np.asarray(inputs["L2"], np.float32),
        np.asarray(inputs["b2"], np.float32), ncols=3)

    key = (KT, nch)
    if key not in _CACHE:
        _CACHE[key] = _build_program(tile_block, nch, chunk_tiles)
    nc = _CACHE[key]

    x0 = np.zeros((Bsz, NP, FEAT), np.float32)
    x0[:, :N, :] = verts
    common = {
        "wcat": wcat.astype(BF16), "lcat": lcat.astype(BF16),
        "bias": bias.astype(BF16), "smat": S, "gidx": gidx_w,
    }
    in_maps = [dict(common, x0=x0[b]) for b in range(Bsz)]
    res = bass_utils.run_bass_kernel_spmd(nc, in_maps, core_ids=list(range(Bsz)))
    out = np.stack([res.results[b]["out"] for b in range(Bsz)], axis=0)
    return out.astype(np.float32)


if __name__ == "__main__":
    sys.path.insert(0, os.path.dirname(os.path.abspath(__file__)))
    import reference as R
    inputs = {k: np.asarray(v) for k, v in R.setup_inputs().items()}
    exp = np.asarray(R.reference(**R.setup_inputs()))
    got = kernel(**inputs)
    err = np.abs(got - exp).max() / np.abs(exp).max()
    print("Relative error:", err)



# revision 22
# speedup vs baseline: 1.8447x; 1.8447x over previous
"""Trainium2 Bass kernel for nn_MeshDeformation (GNN message passing).

Data-parallel over batch B=8 across 8 cores, one batch item per core.

Feature-major design: activations live in SBUF as xT [128 hid-part, 2 ktiles,
NP verts] so no transposes are ever needed:
  - phase M: mm = x@W vertex-major ([v,h] = xT_blk.T @ W) -> bf16 rows to a
    ping-pong HBM buffer. Software-pipelined: conv c+1's phase M is emitted
    inside conv c's gather/scatter loop (block i emitted once conv c finished
    block i, with a small lag so PE never stalls on the evacuation).
  - phase G: batched dma_gather (one SWDGE instruction per CH-tile chunk)
    pulls dst-sorted edge rows into SBUF edge-major; chunks double-buffered.
  - phase S: per 128-edge tile, 2 matmuls (hid halves): pacc_q += g_q.T @ S_t
    giving feature-major psum out [h-half, dst]; the x@L term accumulates into
    the same psum group (lhsT = L quarter, rhs = xT block); bias+ReLU fused
    into the scalar-engine evacuation (activation bias is per-partition =
    per-hid-feature here). Residual folds the *0.5 into the activation scale.
  - S matrices (val folded in) and gather indices stay resident in SBUF for
    all 10 convs.
  - final conv computes y=x@W2 into padded 512B rows, gathers y, single-half
    scatter, Tanh+bias evac, PE-transpose [3,128]->[128,3], x0.1 on evac.
"""
import sys, os
sys.path.insert(0, '/opt/trn_rl_repo')
import numpy as np
import ml_dtypes

import concourse.bass as bass
import concourse.bacc as bacc
import concourse.mybir as mybir
import concourse.tile as tile
from concourse import bass_utils
from concourse.library_config import mlp as _mlp_lib

N = 6890
NP = 6912          # padded vertices (54 * 128)
NB = NP // 128     # 54 dst/vertex blocks
HID = 256
FEAT = 128
NCONV = 10         # conv1, 8 hidden convs, final conv2
CH = 8             # gather tiles per chunk (1024 rows: HW dma_gather limit)
GB = 3             # phase-M blocks per staged HBM write
MLAG = 2           # blocks of lag for interleaved next-conv phase M
DEBUG_STAGE = 0
PIPELINE = True
BIAS_AP = True

BF16 = ml_dtypes.bfloat16


def _edge_tiles(src, dst, val):
    """dst-sorted, per-dst-block 128-padded edge tiling (vectorized).

    Returns (gidx [KT*128] int16 src ids (pad=0), S [KT,128,128] f32,
    tile_block [KT] int array).
    """
    blk = (dst >> 7).astype(np.int64)
    # one gathered row per unique (block, src); duplicate edges fold into S
    pair = blk * 8192 + src                      # unique (block, src) key
    uniq, inv = np.unique(pair, return_inverse=True)
    ublk = (uniq // 8192).astype(np.int64)
    usrc = (uniq % 8192).astype(np.int64)
    counts = np.bincount(ublk, minlength=NB)     # unique rows per block
    ntiles = (counts + 127) // 128
    KT = int(ntiles.sum())
    tile_block = np.repeat(np.arange(NB), ntiles)
    block_start_row = np.concatenate([[0], np.cumsum(counts)])[:-1]
    block_start_tile = np.concatenate([[0], np.cumsum(ntiles)])[:-1]
    nrow = len(uniq)
    within = np.arange(nrow) - block_start_row[ublk]   # row slot within block
    tile_of_row = block_start_tile[ublk] + (within >> 7)
    k_of_row = within & 127
    gidx = np.zeros(KT * 128, np.int16)
    gidx[tile_of_row * 128 + k_of_row] = usrc.astype(np.int16)
    S = np.zeros((KT, 128, 128), np.float32)
    np.add.at(S, (tile_of_row[inv], k_of_row[inv], dst & 127), val)
    return gidx, S, tile_block


def _src_of(c, A, Bb):
    if c == 0:
        return Bb          # x0T in ktile-0 region
    return A if c % 2 == 1 else Bb


def _dst_of(c):
    if c == 0:
        return 'A'
    if c == 9:
        return 'final'
    return 'B' if c % 2 == 1 else 'resid'


def _build_program(tile_block, KT):
    tile_block = list(tile_block)
    chunks = []          # (jlo, nt) with a short ramp so gather-0 lands fast
    j = 0
    for nt in (4, 8):
        if j < KT:
            nt = min(nt, KT - j)
            chunks.append((j, nt))
            j += nt
    while j < KT:
        nt = min(CH, KT - j)
        chunks.append((j, nt))
        j += nt
    nchunks = len(chunks)
    tile_chunk = np.zeros(KT, np.int64)
    for ci, (jlo, nt) in enumerate(chunks):
        tile_chunk[jlo:jlo + nt] = ci
    tiles_of = {}
    for j, b in enumerate(tile_block):
        tiles_of.setdefault(b, []).append(j)

    nc = bacc.Bacc("TRN2", target_bir_lowering=False, debug=False)
    bf = mybir.dt.bfloat16
    f32 = mybir.dt.float32

    x0_d = nc.dram_tensor("x0T", [128, NP], bf, kind="ExternalInput")
    wcat_d = nc.dram_tensor("wcat", [128, NCONV * 2 * HID], bf,
                            kind="ExternalInput")
    lcq_d = nc.dram_tensor("lcq", [128, NCONV * 4 * 128], bf,
                           kind="ExternalInput")
    bcol_d = nc.dram_tensor("bcol", [128, NCONV * 4], f32,
                            kind="ExternalInput")
    s_d = nc.dram_tensor("smat", [128, KT * 128], bf, kind="ExternalInput")
    gidx_d = nc.dram_tensor("gidx", [128, KT * 8], mybir.dt.int16,
                            kind="ExternalInput")
    out_d = nc.dram_tensor("out", [N, 3], f32, kind="ExternalOutput")
    if DEBUG_STAGE:
        dbg_d = nc.dram_tensor("dbg", [128, 2 * NP], bf, kind="ExternalOutput")

    from concourse.masks import make_identity

    with tile.TileContext(nc) as tc:
        with (
            tc.tile_pool(name="dram", bufs=1, space="DRAM") as dram,
            tc.tile_pool(name="res", bufs=1) as res,
            tc.tile_pool(name="gpool", bufs=4) as gpool,
            tc.tile_pool(name="stg", bufs=3) as stg,
            tc.tile_pool(name="rstg", bufs=4) as rstg,
            tc.tile_pool(name="pmp", bufs=3, space="PSUM") as pmp,
            tc.tile_pool(name="acc", bufs=3, space="PSUM") as acc,
            tc.tile_pool(name="ptp", bufs=2, space="PSUM") as ptp,
        ):
            mm_a = dram.tile([NP, HID], bf, tag="mm0")
            mm_b = dram.tile([NP, HID], bf, tag="mm1")
            mm_pp = [mm_a, mm_b]

            S_all = res.tile([128, KT * 128], bf, tag="S")
            gix = res.tile([128, KT * 8], mybir.dt.int16, tag="gix")
            A = res.tile([128, 2 * NP], bf, tag="A")
            Bb = res.tile([128, 2 * NP], bf, tag="B")
            wc = res.tile([128, NCONV * 2 * HID], bf, tag="wc")
            lcq = res.tile([128, NCONV * 4 * 128], bf, tag="lcq")
            bcol = res.tile([128, NCONV * 4], f32, tag="bcol")
            id32 = res.tile([128, 128], f32, tag="id32")
            idbf = res.tile([128, 128], bf, tag="idbf")
            ostage = res.tile([128, NB * 3], f32, tag="ostage")
            snt = res.tile([128, 8], f32, tag="snt")
            sdm = res.tile([128, 8], f32, tag="sdm")

            nc.gpsimd.load_library(_mlp_lib)
            nc.sync.dma_start(out=wc[:], in_=wcat_d[:])
            nc.sync.dma_start(out=Bb[:, 0:NP], in_=x0_d[:])
            nc.sync.dma_start(out=lcq[:], in_=lcq_d[:])
            nc.sync.dma_start(out=bcol[:], in_=bcol_d[:])
            nc.sync.dma_start(out=gix[:], in_=gidx_d[:])
            make_identity(nc, id32[:])
            nc.vector.tensor_copy(out=idbf[:], in_=id32[:])

            nregs = {}
            for (_, nt) in chunks:
                if nt not in nregs:
                    nregs[nt] = nc.gpsimd.to_reg(nt * 128)

            def xs(src_tile, h, i):
                return src_tile[:, h * NP + i * 128: h * NP + (i + 1) * 128]

            # ---- phase M emitter (per-block, staged writes of GB blocks) ----
            def m_state(c, src_tile, mm_d):
                return dict(c=c, src=src_tile, mm=mm_d, ms=None, base=0, cnt=0)

            def emit_m_block(st, i):
                c, src_tile, mm_d = st['c'], st['src'], st['mm']
                fin = 1 if c == 0 else 2
                pm = pmp.tile([128, HID], f32, tag="pm", name="pm")
                for h in range(fin):
                    nc.tensor.matmul(
                        out=pm[:], lhsT=xs(src_tile, h, i),
                        rhs=wc[:, (2 * c + h) * HID:(2 * c + h + 1) * HID],
                        start=(h == 0), stop=(h == fin - 1))
                if st['cnt'] == 0:
                    st['ms'] = stg.tile([128, GB * HID], bf, tag="ms",
                                        name="ms")
                    st['base'] = i
                sl = st['cnt']
                if i % 2 == 0:
                    nc.vector.tensor_copy(
                        out=st['ms'][:, sl * HID:(sl + 1) * HID], in_=pm[:])
                else:
                    nc.scalar.copy(
                        out=st['ms'][:, sl * HID:(sl + 1) * HID], in_=pm[:])
                st['cnt'] += 1
                if st['cnt'] == GB or i == NB - 1:
                    lo, nb = st['base'], st['cnt']
                    nc.sync.dma_start(
                        out=mm_d[lo * 128:(lo + nb) * 128, :].rearrange(
                            "(b p) h -> p b h", p=128),
                        in_=st['ms'][:, :nb * HID].rearrange(
                            "p (b h) -> p b h", h=HID))
                    st['cnt'] = 0
                if i == NB - 1:
                    nc.sync.dma_start(out=snt[:], in_=bcol_d[:, 0:8])

            # ---- gather + scatter phase for one conv ----
            def conv_gs(c, src_tile, dst_mode, mm_d, nxt):
                fin = 1 if c == 0 else 2
                final = dst_mode == 'final'
                nq = 1 if final else 2

                gtiles = {}
                issued = [0]
                fpend = []
                # final conv: y lives in cols 0:128 of the 256-wide mm rows;
                # gather only 256B per row
                gel = 128 if final else HID

                def need_chunk(k):
                    while issued[0] <= min(k + 1, nchunks - 1):
                        ci = issued[0]
                        jlo, nt = chunks[ci]
                        if c == 0:
                            # stream the resident S in step with conv0's use
                            nc.sync.dma_start(
                                out=S_all[:, jlo * 128:(jlo + nt) * 128],
                                in_=s_d[:, jlo * 128:(jlo + nt) * 128])
                        gt = gpool.tile([128, CH * HID], bf, tag="g", name="g")
                        nc.gpsimd.dma_gather(
                            gt[:, :nt * gel].rearrange("p (t e) -> p t e",
                                                       e=gel),
                            mm_d[:, 0:gel], gix[:, jlo * 8:(jlo + nt) * 8],
                            nt * 128, nregs[nt], gel,
                            elem_step=HID)
                        gtiles[ci] = gt
                        issued[0] += 1
                    return gtiles[k]

                def lterm(i, pacc2, has_tiles):
                    for q in range(nq):
                        for h in range(fin):
                            nc.tensor.matmul(
                                out=pacc2[q][:],
                                lhsT=lcq[:, (c * 4 + h * 2 + q) * 128:
                                         (c * 4 + h * 2 + q + 1) * 128],
                                rhs=xs(src_tile, h, i),
                                start=(h == 0),
                                stop=(h == fin - 1) and not has_tiles)

                def finish(i, pacc2, started):
                    for q in range(nq):
                        pq = pacc2[q][:]
                        if dst_mode in ('A', 'B'):
                            dbuf = A if dst_mode == 'A' else Bb
                            nc.scalar.activation(
                                out=dbuf[:, q * NP + i * 128:
                                         q * NP + (i + 1) * 128],
                                in_=pq,
                                func=mybir.ActivationFunctionType.Relu,
                                bias=(bcol[:, c * 4 + q: c * 4 + q + 1]
                                      if BIAS_AP else 0.0))
                        elif dst_mode == 'resid':
                            # A = 0.5*A + relu(0.5*pacc + 0.5*b)
                            asl = A[:, q * NP + i * 128: q * NP + (i + 1) * 128]
                            t = rstg.tile([128, 128], bf, tag="rt", name="rt")
                            nc.scalar.activation(
                                out=t[:], in_=pq,
                                func=mybir.ActivationFunctionType.Relu,
                                scale=0.5,
                                bias=bcol[:, c * 4 + 2 + q: c * 4 + 2 + q + 1])
                            nc.vector.tensor_scalar_mul(asl, asl, 0.5)
                            nc.vector.tensor_tensor(
                                out=asl, in0=asl, in1=t[:],
                                op=mybir.AluOpType.add)
                        else:  # final
                            t = rstg.tile([128, 128], bf, tag="tt", name="tt")
                            nc.scalar.activation(
                                out=t[0:3, :], in_=pacc2[0][0:3, :],
                                func=mybir.ActivationFunctionType.Tanh,
                                bias=bcol[0:3, c * 4: c * 4 + 1])
                            fpend.append((i, t))

                def flush_final(i):
                    while fpend and fpend[0][0] <= i:
                        fi, t = fpend.pop(0)
                        pt = ptp.tile([128, 128], bf, tag="pt", name="pt")
                        nc.tensor.transpose(
                            out=pt[:, 0:3], in_=t[0:3, :],
                            identity=idbf[0:3, 0:3])
                        nc.scalar.mul(
                            out=ostage[:, fi * 3:(fi + 1) * 3],
                            in_=pt[:, 0:3], mul=0.1)

                for i in range(NB):
                    pacc2 = [acc.tile([128, 128], f32, tag="pacc", name="pacc")
                             for _ in range(nq)]
                    tj = tiles_of.get(i, [])
                    lterm(i, pacc2, bool(tj))
                    for j in tj:
                        k = int(tile_chunk[j])
                        gt = need_chunk(k)
                        jj = j - chunks[k][0]
                        last = j == tj[-1]
                        for q in range(nq):
                            nc.tensor.matmul(
                                out=pacc2[q][:],
                                lhsT=gt[:, jj * gel + q * 128:
                                        jj * gel + (q + 1) * 128],
                                rhs=S_all[:, j * 128:(j + 1) * 128],
                                start=False, stop=last)
                    finish(i, pacc2, [True, True])
                    if final:
                        flush_final(i - MLAG)
                    if nxt is not None and i >= MLAG:
                        emit_m_block(nxt, i - MLAG)
                if nxt is not None:
                    for i in range(NB - MLAG, NB):
                        emit_m_block(nxt, i)
                if final:
                    flush_final(NB)

                if final:
                    nfull = N // 128  # 53 full blocks
                    nc.gpsimd.dma_start(
                        out=out_d[0:nfull * 128, :].rearrange(
                            "(i p) c -> p i c", p=128),
                        in_=ostage[:, 0:nfull * 3].rearrange(
                            "p (i c) -> p i c", c=3))
                    rem = N - nfull * 128
                    nc.gpsimd.dma_start(
                        out=out_d[nfull * 128:N, :],
                        in_=ostage[0:rem, nfull * 3:(nfull + 1) * 3])

            # ---- network ----
            ncv = 10 if DEBUG_STAGE == 0 else {1: 1, 2: 2, 3: 3, 9: 9}[DEBUG_STAGE]
            states = [None] * (NCONV + 1)
            states[0] = m_state(0, _src_of(0, A, Bb), mm_pp[0])
            for i in range(NB):
                emit_m_block(states[0], i)
            for c in range(ncv):
                # Pool-issued SBUF->SBUF DMA reading the sentinel: Q7 waits
                # for the sentinel HWDGE write (FIFO after all mm writes), so
                # every later gather sees a fully-written mm buffer.
                nc.gpsimd.dma_start(out=sdm[:], in_=snt[:])
                nxt = None
                if c + 1 < ncv:
                    states[c + 1] = m_state(c + 1, _src_of(c + 1, A, Bb),
                                            mm_pp[(c + 1) % 2])
                    if PIPELINE:
                        nxt = states[c + 1]
                conv_gs(c, _src_of(c, A, Bb), _dst_of(c), mm_pp[c % 2], nxt)
                if not PIPELINE and c + 1 < ncv:
                    for i in range(NB):
                        emit_m_block(states[c + 1], i)
            if DEBUG_STAGE:
                dsrc = A if DEBUG_STAGE in (1, 3, 9) else Bb
                nc.sync.dma_start(out=dbg_d[:], in_=dsrc[:])

    nc.finalize()
    return nc


_CACHE = {}
TRACE = False
LAST_RESULTS = None


def _host_arrays(inputs):
    src = np.asarray(inputs["edge_src"]).astype(np.int64)
    dst = np.asarray(inputs["edge_dst"]).astype(np.int64)
    val = np.asarray(inputs["edge_val"], np.float32)

    gidx, S, tile_block = _edge_tiles(src, dst, val)
    KT = len(tile_block)
    s_host = np.ascontiguousarray(
        S.transpose(1, 0, 2).reshape(128, KT * 128)).astype(BF16)
    gidx_w = np.ascontiguousarray(
        np.tile(gidx.reshape(KT * 8, 16).T, (8, 1)))

    wcat = np.zeros((128, NCONV * 2 * HID), np.float32)
    lcq = np.zeros((128, NCONV * 4 * 128), np.float32)
    # bcol layout per conv c: col c*4+q = b[q-half]; col c*4+2+q = 0.5*b
    bcol = np.zeros((128, NCONV * 4), np.float32)

    def put(c, W, L, b):
        nh = W.shape[0] // 128
        no = W.shape[1]
        for h in range(nh):
            wcat[:, (2 * c + h) * HID:(2 * c + h) * HID + no] = \
                W[h * 128:(h + 1) * 128]
            for q in range(2):
                qs = slice(q * 128, min((q + 1) * 128, no))
                ncol = qs.stop - qs.start
                if ncol <= 0:
                    continue
                lcq[:, (c * 4 + h * 2 + q) * 128:
                    (c * 4 + h * 2 + q) * 128 + ncol] = \
                    L[h * 128:(h + 1) * 128, qs]
        for q in range(2):
            qs = slice(q * 128, min((q + 1) * 128, len(b)))
            ncol = qs.stop - qs.start
            if ncol <= 0:
                continue
            bcol[0:ncol, c * 4 + q] = b[qs]
            bcol[0:ncol, c * 4 + 2 + q] = 0.5 * b[qs]

    put(0, np.asarray(inputs["W1"], np.float32),
        np.asarray(inputs["L1"], np.float32),
        np.asarray(inputs["b1"], np.float32))
    Wb = np.asarray(inputs["Wb"], np.float32)
    Lb = np.asarray(inputs["Lb"], np.float32)
    bb = np.asarray(inputs["bb"], np.float32)
    for k in range(8):
        put(1 + k, Wb[k], Lb[k], bb[k])
    put(9, np.asarray(inputs["W2"], np.float32),
        np.asarray(inputs["L2"], np.float32),
        np.asarray(inputs["b2"], np.float32))

    common = {
        "wcat": wcat.astype(BF16), "lcq": lcq.astype(BF16),
        "bcol": bcol, "smat": s_host, "gidx": gidx_w,
    }
    return common, tile_block, KT


def kernel(**inputs):
    verts = np.asarray(inputs["verts_feats"], np.float32)   # [8, 6890, 128]
    Bsz = verts.shape[0]
    common, tile_block, KT = _host_arrays(inputs)

    key = (KT, tuple(tile_block))
    if key not in _CACHE:
        _CACHE.clear()
        _CACHE[key] = _build_program(tile_block, KT)
    nc = _CACHE[key]

    x0T = np.zeros((Bsz, 128, NP), np.float32)
    x0T[:, :, :N] = verts.transpose(0, 2, 1)
    x0T = x0T.astype(BF16)
    in_maps = [dict(common, x0T=x0T[b]) for b in range(Bsz)]
    kw = dict(trace=True) if TRACE else {}
    res = bass_utils.run_bass_kernel_spmd(nc, in_maps, core_ids=list(range(Bsz)),
                                          **kw)
    if TRACE:
        globals()['LAST_RESULTS'] = res
    out = np.stack([res.results[b]["out"] for b in range(Bsz)], axis=0)
    return out.astype(np.float32)


if __name__ == "__main__":
    sys.path.insert(0, os.path.dirname(os.path.abspath(__file__)))
    import reference as R
    inputs = {k: np.asarray(v) for k, v in R.setup_inputs().items()}
    exp = np.asarray(R.reference(**R.setup_inputs()))
    got = kernel(**inputs)
    err = np.abs(got - exp).max() / np.abs(exp).max()
    print("Relative error:", err)


# revision 24
# speedup vs baseline: 2.8028x; 1.5194x over previous
"""Trainium2 Bass kernel for nn_MeshDeformation (GNN message passing).

Data-parallel over batch B=8 across 8 cores, one batch item per core.

Feature-major design: activations live in SBUF as xT [128 hid-part, 2 ktiles,
NP verts] so no transposes are ever needed:
  - phase M: mm = x@W vertex-major ([v,h] = xT_blk.T @ W) -> bf16 rows to a
    ping-pong HBM buffer. Software-pipelined: conv c+1's phase M is emitted
    inside conv c's gather/scatter loop (block i emitted once conv c finished
    block i, with a small lag so PE never stalls on the evacuation).
  - phase G: batched dma_gather (one SWDGE instruction per CH-tile chunk)
    pulls dst-sorted edge rows into SBUF edge-major; chunks double-buffered.
  - phase S: per 128-edge tile, 2 matmuls (hid halves): pacc_q += g_q.T @ S_t
    giving feature-major psum out [h-half, dst]; the x@L term accumulates into
    the same psum group (lhsT = L quarter, rhs = xT block); bias+ReLU fused
    into the scalar-engine evacuation (activation bias is per-partition =
    per-hid-feature here). Residual folds the *0.5 into the activation scale.
  - S matrices (val folded in) and gather indices stay resident in SBUF for
    all 10 convs.
  - final conv computes y=x@W2 into padded 512B rows, gathers y, single-half
    scatter, Tanh+bias evac, PE-transpose [3,128]->[128,3], x0.1 on evac.
"""
import sys, os
sys.path.insert(0, '/opt/trn_rl_repo')
import numpy as np
import ml_dtypes

import concourse.bass as bass
import concourse.bacc as bacc
import concourse.mybir as mybir
import concourse.tile as tile
from concourse import bass_utils
from concourse.library_config import mlp as _mlp_lib

N = 6890
NP = 6912          # padded vertices (54 * 128)
NB = NP // 128     # 54 dst/vertex blocks
HID = 256
FEAT = 128
NCONV = 10         # conv1, 8 hidden convs, final conv2
CH = 8             # gather tiles per chunk (1024 rows: HW dma_gather limit)
GB = 3             # phase-M blocks per staged HBM write
MLAG = 2           # blocks of lag for interleaved next-conv phase M
DEBUG_STAGE = 0
PIPELINE = True
BIAS_AP = True

BF16 = ml_dtypes.bfloat16


def _edge_tiles(src, dst, val):
    """dst-sorted, per-dst-block 128-padded edge tiling (vectorized).

    Returns (gidx [KT*128] int16 src ids (pad=0), S [KT,128,128] f32,
    tile_block [KT] int array).
    """
    blk = (dst >> 7).astype(np.int64)
    # one gathered row per unique (block, src); duplicate edges fold into S
    pair = blk * 8192 + src                      # unique (block, src) key
    uniq, inv = np.unique(pair, return_inverse=True)
    ublk = (uniq // 8192).astype(np.int64)
    usrc = (uniq % 8192).astype(np.int64)
    counts = np.bincount(ublk, minlength=NB)     # unique rows per block
    ntiles = (counts + 127) // 128
    KT = int(ntiles.sum())
    tile_block = np.repeat(np.arange(NB), ntiles)
    block_start_row = np.concatenate([[0], np.cumsum(counts)])[:-1]
    block_start_tile = np.concatenate([[0], np.cumsum(ntiles)])[:-1]
    nrow = len(uniq)
    within = np.arange(nrow) - block_start_row[ublk]   # row slot within block
    tile_of_row = block_start_tile[ublk] + (within >> 7)
    k_of_row = within & 127
    gidx = np.zeros(KT * 128, np.int16)
    gidx[tile_of_row * 128 + k_of_row] = usrc.astype(np.int16)
    S = np.zeros((KT, 128, 128), np.float32)
    np.add.at(S, (tile_of_row[inv], k_of_row[inv], dst & 127), val)
    return gidx, S, tile_block


def _src_of(c, A, Bb):
    if c == 0:
        return Bb          # x0T in ktile-0 region
    return A if c % 2 == 1 else Bb


def _dst_of(c):
    if c == 0:
        return 'A'
    if c == 9:
        return 'final'
    return 'B' if c % 2 == 1 else 'resid'


def _build_program(tile_block, KT):
    tile_block = list(tile_block)
    chunks = []          # (jlo, nt) with a short ramp so gather-0 lands fast
    j = 0
    for nt in (4, 8):
        if j < KT:
            nt = min(nt, KT - j)
            chunks.append((j, nt))
            j += nt
    while j < KT:
        nt = min(CH, KT - j)
        chunks.append((j, nt))
        j += nt
    nchunks = len(chunks)
    tile_chunk = np.zeros(KT, np.int64)
    for ci, (jlo, nt) in enumerate(chunks):
        tile_chunk[jlo:jlo + nt] = ci
    tiles_of = {}
    for j, b in enumerate(tile_block):
        tiles_of.setdefault(b, []).append(j)

    nc = bacc.Bacc("TRN2", target_bir_lowering=False, debug=False)
    bf = mybir.dt.bfloat16
    f32 = mybir.dt.float32

    x0_d = nc.dram_tensor("x0T", [128, NP], bf, kind="ExternalInput")
    wcat_d = nc.dram_tensor("wcat", [128, NCONV * 2 * HID], bf,
                            kind="ExternalInput")
    lcq_d = nc.dram_tensor("lcq", [128, NCONV * 4 * 128], bf,
                           kind="ExternalInput")
    bcol_d = nc.dram_tensor("bcol", [128, NCONV * 4], f32,
                            kind="ExternalInput")
    s_d = nc.dram_tensor("smat", [128, KT * 128], bf, kind="ExternalInput")
    gidx_d = nc.dram_tensor("gidx", [128, KT * 8], mybir.dt.int16,
                            kind="ExternalInput")
    out_d = nc.dram_tensor("out", [N, 3], f32, kind="ExternalOutput")
    if DEBUG_STAGE:
        dbg_d = nc.dram_tensor("dbg", [128, 2 * NP], bf, kind="ExternalOutput")

    from concourse.masks import make_identity

    with tile.TileContext(nc) as tc:
        with (
            tc.tile_pool(name="dram", bufs=1, space="DRAM") as dram,
            tc.tile_pool(name="res", bufs=1) as res,
            tc.tile_pool(name="gpool", bufs=4) as gpool,
            tc.tile_pool(name="stg", bufs=3) as stg,
            tc.tile_pool(name="rstg", bufs=4) as rstg,
            tc.tile_pool(name="pmp", bufs=3, space="PSUM") as pmp,
            tc.tile_pool(name="acc", bufs=3, space="PSUM") as acc,
            tc.tile_pool(name="ptp", bufs=2, space="PSUM") as ptp,
        ):
            mm_a = dram.tile([NP, HID], bf, tag="mm0")
            mm_b = dram.tile([NP, HID], bf, tag="mm1")
            mm_pp = [mm_a, mm_b]

            S_all = res.tile([128, KT * 128], bf, tag="S")
            gix = res.tile([128, KT * 8], mybir.dt.int16, tag="gix")
            A = res.tile([128, 2 * NP], bf, tag="A")
            Bb = res.tile([128, 2 * NP], bf, tag="B")
            wc = res.tile([128, NCONV * 2 * HID], bf, tag="wc")
            lcq = res.tile([128, NCONV * 4 * 128], bf, tag="lcq")
            bcol = res.tile([128, NCONV * 4], f32, tag="bcol")
            id32 = res.tile([128, 128], f32, tag="id32")
            idbf = res.tile([128, 128], bf, tag="idbf")
            ostage = res.tile([128, NB * 3], f32, tag="ostage")
            snt = res.tile([128, 8], f32, tag="snt")
            sdm = res.tile([128, 8], f32, tag="sdm")

            nc.gpsimd.load_library(_mlp_lib)
            nc.sync.dma_start(out=wc[:], in_=wcat_d[:])
            nc.sync.dma_start(out=Bb[:, 0:NP], in_=x0_d[:])
            nc.sync.dma_start(out=lcq[:], in_=lcq_d[:])
            nc.sync.dma_start(out=bcol[:], in_=bcol_d[:])
            nc.sync.dma_start(out=gix[:], in_=gidx_d[:])
            make_identity(nc, id32[:])
            nc.vector.tensor_copy(out=idbf[:], in_=id32[:])

            nregs = {}
            for (_, nt) in chunks:
                if nt not in nregs:
                    nregs[nt] = nc.gpsimd.to_reg(nt * 128)

            def xs(src_tile, h, i):
                return src_tile[:, h * NP + i * 128: h * NP + (i + 1) * 128]

            # ---- phase M emitter (per-block, staged writes of GB blocks) ----
            def m_state(c, src_tile, mm_d):
                return dict(c=c, src=src_tile, mm=mm_d, ms=None, base=0, cnt=0)

            def emit_m_block(st, i):
                c, src_tile, mm_d = st['c'], st['src'], st['mm']
                fin = 1 if c == 0 else 2
                pm = pmp.tile([128, HID], f32, tag="pm", name="pm")
                for h in range(fin):
                    nc.tensor.matmul(
                        out=pm[:], lhsT=xs(src_tile, h, i),
                        rhs=wc[:, (2 * c + h) * HID:(2 * c + h + 1) * HID],
                        start=(h == 0), stop=(h == fin - 1))
                if st['cnt'] == 0:
                    st['ms'] = stg.tile([128, GB * HID], bf, tag="ms",
                                        name="ms")
                    st['base'] = i
                sl = st['cnt']
                if i % 2 == 0:
                    nc.vector.tensor_copy(
                        out=st['ms'][:, sl * HID:(sl + 1) * HID], in_=pm[:])
                else:
                    nc.scalar.copy(
                        out=st['ms'][:, sl * HID:(sl + 1) * HID], in_=pm[:])
                st['cnt'] += 1
                if st['cnt'] == GB or i == NB - 1:
                    lo, nb = st['base'], st['cnt']
                    nc.sync.dma_start(
                        out=mm_d[lo * 128:(lo + nb) * 128, :].rearrange(
                            "(b p) h -> p b h", p=128),
                        in_=st['ms'][:, :nb * HID].rearrange(
                            "p (b h) -> p b h", h=HID))
                    st['cnt'] = 0
                if i == NB - 1:
                    nc.sync.dma_start(out=snt[:], in_=bcol_d[:, 0:8])

            # ---- gather + scatter phase for one conv ----
            def conv_gs(c, src_tile, dst_mode, mm_d, nxt):
                fin = 1 if c == 0 else 2
                final = dst_mode == 'final'
                nq = 1 if final else 2

                gtiles = {}
                issued = [0]
                fpend = []
                # final conv: y lives in cols 0:128 of the 256-wide mm rows;
                # gather only 256B per row
                gel = 128 if final else HID

                def need_chunk(k):
                    while issued[0] <= min(k + 1, nchunks - 1):
                        ci = issued[0]
                        jlo, nt = chunks[ci]
                        if c == 0:
                            # stream the resident S in step with conv0's use
                            nc.sync.dma_start(
                                out=S_all[:, jlo * 128:(jlo + nt) * 128],
                                in_=s_d[:, jlo * 128:(jlo + nt) * 128])
                        gt = gpool.tile([128, CH * HID], bf, tag="g", name="g")
                        nc.gpsimd.dma_gather(
                            gt[:, :nt * gel].rearrange("p (t e) -> p t e",
                                                       e=gel),
                            mm_d[:, 0:gel], gix[:, jlo * 8:(jlo + nt) * 8],
                            nt * 128, nregs[nt], gel,
                            elem_step=HID)
                        gtiles[ci] = gt
                        issued[0] += 1
                    return gtiles[k]

                def lterm(i, pacc2, has_tiles):
                    for q in range(nq):
                        for h in range(fin):
                            nc.tensor.matmul(
                                out=pacc2[q][:],
                                lhsT=lcq[:, (c * 4 + h * 2 + q) * 128:
                                         (c * 4 + h * 2 + q + 1) * 128],
                                rhs=xs(src_tile, h, i),
                                start=(h == 0),
                                stop=(h == fin - 1) and not has_tiles)

                def finish(i, pacc2, started):
                    for q in range(nq):
                        pq = pacc2[q][:]
                        if dst_mode in ('A', 'B'):
                            dbuf = A if dst_mode == 'A' else Bb
                            nc.scalar.activation(
                                out=dbuf[:, q * NP + i * 128:
                                         q * NP + (i + 1) * 128],
                                in_=pq,
                                func=mybir.ActivationFunctionType.Relu,
                                bias=(bcol[:, c * 4 + q: c * 4 + q + 1]
                                      if BIAS_AP else 0.0))
                        elif dst_mode == 'resid':
                            # A = 0.5*A + relu(0.5*pacc + 0.5*b)
                            asl = A[:, q * NP + i * 128: q * NP + (i + 1) * 128]
                            t = rstg.tile([128, 128], bf, tag="rt", name="rt")
                            nc.scalar.activation(
                                out=t[:], in_=pq,
                                func=mybir.ActivationFunctionType.Relu,
                                scale=0.5,
                                bias=bcol[:, c * 4 + 2 + q: c * 4 + 2 + q + 1])
                            nc.vector.tensor_scalar_mul(asl, asl, 0.5)
                            nc.vector.tensor_tensor(
                                out=asl, in0=asl, in1=t[:],
                                op=mybir.AluOpType.add)
                        else:  # final
                            t = rstg.tile([128, 128], bf, tag="tt", name="tt")
                            nc.scalar.activation(
                                out=t[0:3, :], in_=pacc2[0][0:3, :],
                                func=mybir.ActivationFunctionType.Tanh,
                                bias=bcol[0:3, c * 4: c * 4 + 1])
                            fpend.append((i, t))

                def flush_final(i):
                    while fpend and fpend[0][0] <= i:
                        fi, t = fpend.pop(0)
                        pt = ptp.tile([128, 128], bf, tag="pt", name="pt")
                        nc.tensor.transpose(
                            out=pt[:, 0:3], in_=t[0:3, :],
                            identity=idbf[0:3, 0:3])
                        nc.scalar.mul(
                            out=ostage[:, fi * 3:(fi + 1) * 3],
                            in_=pt[:, 0:3], mul=0.1)

                for i in range(NB):
                    pacc2 = [acc.tile([128, 128], f32, tag="pacc", name="pacc")
                             for _ in range(nq)]
                    tj = tiles_of.get(i, [])
                    lterm(i, pacc2, bool(tj))
                    for j in tj:
                        k = int(tile_chunk[j])
                        gt = need_chunk(k)
                        jj = j - chunks[k][0]
                        last = j == tj[-1]
                        for q in range(nq):
                            nc.tensor.matmul(
                                out=pacc2[q][:],
                                lhsT=gt[:, jj * gel + q * 128:
                                        jj * gel + (q + 1) * 128],
                                rhs=S_all[:, j * 128:(j + 1) * 128],
                                start=False, stop=last)
                    finish(i, pacc2, [True, True])
                    if final:
                        flush_final(i - MLAG)
                    if nxt is not None and i >= MLAG:
                        emit_m_block(nxt, i - MLAG)
                if nxt is not None:
                    for i in range(NB - MLAG, NB):
                        emit_m_block(nxt, i)
                if final:
                    flush_final(NB)

                if final:
                    nfull = N // 128  # 53 full blocks
                    nc.gpsimd.dma_start(
                        out=out_d[0:nfull * 128, :].rearrange(
                            "(i p) c -> p i c", p=128),
                        in_=ostage[:, 0:nfull * 3].rearrange(
                            "p (i c) -> p i c", c=3))
                    rem = N - nfull * 128
                    nc.gpsimd.dma_start(
                        out=out_d[nfull * 128:N, :],
                        in_=ostage[0:rem, nfull * 3:(nfull + 1) * 3])

            # ---- network ----
            ncv = 10 if DEBUG_STAGE == 0 else {1: 1, 2: 2, 3: 3, 9: 9}[DEBUG_STAGE]
            states = [None] * (NCONV + 1)
            states[0] = m_state(0, _src_of(0, A, Bb), mm_pp[0])
            for i in range(NB):
                emit_m_block(states[0], i)
            for c in range(ncv):
                # Pool-issued SBUF->SBUF DMA reading the sentinel: Q7 waits
                # for the sentinel HWDGE write (FIFO after all mm writes), so
                # every later gather sees a fully-written mm buffer.
                nc.gpsimd.dma_start(out=sdm[:], in_=snt[:])
                nxt = None
                if c + 1 < ncv:
                    states[c + 1] = m_state(c + 1, _src_of(c + 1, A, Bb),
                                            mm_pp[(c + 1) % 2])
                    if PIPELINE:
                        nxt = states[c + 1]
                conv_gs(c, _src_of(c, A, Bb), _dst_of(c), mm_pp[c % 2], nxt)
                if not PIPELINE and c + 1 < ncv:
                    for i in range(NB):
                        emit_m_block(states[c + 1], i)
            if DEBUG_STAGE:
                dsrc = A if DEBUG_STAGE in (1, 3, 9) else Bb
                nc.sync.dma_start(out=dbg_d[:], in_=dsrc[:])

    nc.finalize()
    return nc


_CACHE = {}
TRACE = False
LAST_RESULTS = None


def _make_dispatch(nc, n_cores):
    """Build a cached PJRT dispatcher (mirrors bass2jax.run_bass_via_pjrt but
    traces/compiles the jitted callable once instead of per call)."""
    import jax
    from jax.sharding import Mesh, PartitionSpec
    from jax.experimental.shard_map import shard_map
    from concourse import bass2jax
    import concourse.mybir as mb

    bass2jax.install_neuronx_cc_hook()
    partition_name = (nc.partition_id_tensor.name
                      if nc.partition_id_tensor else None)
    in_names, out_names, out_avals, zero_outs = [], [], [], []
    for alloc in nc.m.functions[0].allocations:
        if not isinstance(alloc, mb.MemoryLocationSet):
            continue
        name = alloc.memorylocations[0].name
        if alloc.kind == "ExternalInput":
            if name != partition_name:
                in_names.append(name)
        elif alloc.kind == "ExternalOutput":
            shape = tuple(alloc.tensor_shape)
            dtype = mb.dt.np(alloc.dtype)
            out_names.append(name)
            out_avals.append(jax.core.ShapedArray(shape, dtype))
            zero_outs.append(np.zeros(shape, dtype))
    n_params = len(in_names)
    n_outs = len(out_avals)
    all_in_names = list(in_names) + list(out_names)
    if partition_name is not None:
        all_in_names.append(partition_name)
    donate = tuple(range(n_params, n_params + n_outs))

    def _body(*args):
        operands = list(args)
        if partition_name is not None:
            operands.append(bass2jax.partition_id_tensor())
        outs = bass2jax._bass_exec_p.bind(
            *operands,
            out_avals=tuple(out_avals),
            in_names=tuple(all_in_names),
            out_names=tuple(out_names),
            lowering_input_output_aliases=(),
            sim_require_finite=True,
            sim_require_nnan=True,
            nc=nc,
        )
        return tuple(outs)

    devices = jax.devices()[:n_cores]
    mesh = Mesh(np.asarray(devices), ("core",))
    in_specs = (PartitionSpec("core"),) * (n_params + n_outs)
    out_specs = (PartitionSpec("core"),) * n_outs
    sharded = jax.jit(
        shard_map(_body, mesh=mesh, in_specs=in_specs, out_specs=out_specs,
                  check_rep=False),
        donate_argnums=donate, keep_unused=True)

    def run(in_maps):
        per_core = [[np.asarray(m[name]) for name in in_names]
                    for m in in_maps]
        concat_in = [
            np.concatenate([per_core[c][i] for c in range(n_cores)], axis=0)
            for i in range(n_params)]
        concat_zeros = [
            np.zeros((n_cores * z.shape[0], *z.shape[1:]), z.dtype)
            for z in zero_outs]
        out_arrs = sharded(*concat_in, *concat_zeros)
        return [
            {name: np.asarray(out_arrs[i]).reshape(
                n_cores, *out_avals[i].shape)[c]
             for i, name in enumerate(out_names)}
            for c in range(n_cores)]

    return run


def _host_arrays(inputs):
    src = np.asarray(inputs["edge_src"]).astype(np.int64)
    dst = np.asarray(inputs["edge_dst"]).astype(np.int64)
    val = np.asarray(inputs["edge_val"], np.float32)

    gidx, S, tile_block = _edge_tiles(src, dst, val)
    KT = len(tile_block)
    s_host = np.ascontiguousarray(
        S.transpose(1, 0, 2).reshape(128, KT * 128)).astype(BF16)
    gidx_w = np.ascontiguousarray(
        np.tile(gidx.reshape(KT * 8, 16).T, (8, 1)))

    wcat = np.zeros((128, NCONV * 2 * HID), np.float32)
    lcq = np.zeros((128, NCONV * 4 * 128), np.float32)
    # bcol layout per conv c: col c*4+q = b[q-half]; col c*4+2+q = 0.5*b
    bcol = np.zeros((128, NCONV * 4), np.float32)

    def put(c, W, L, b):
        nh = W.shape[0] // 128
        no = W.shape[1]
        for h in range(nh):
            wcat[:, (2 * c + h) * HID:(2 * c + h) * HID + no] = \
                W[h * 128:(h + 1) * 128]
            for q in range(2):
                qs = slice(q * 128, min((q + 1) * 128, no))
                ncol = qs.stop - qs.start
                if ncol <= 0:
                    continue
                lcq[:, (c * 4 + h * 2 + q) * 128:
                    (c * 4 + h * 2 + q) * 128 + ncol] = \
                    L[h * 128:(h + 1) * 128, qs]
        for q in range(2):
            qs = slice(q * 128, min((q + 1) * 128, len(b)))
            ncol = qs.stop - qs.start
            if ncol <= 0:
                continue
            bcol[0:ncol, c * 4 + q] = b[qs]
            bcol[0:ncol, c * 4 + 2 + q] = 0.5 * b[qs]

    put(0, np.asarray(inputs["W1"], np.float32),
        np.asarray(inputs["L1"], np.float32),
        np.asarray(inputs["b1"], np.float32))
    Wb = np.asarray(inputs["Wb"], np.float32)
    Lb = np.asarray(inputs["Lb"], np.float32)
    bb = np.asarray(inputs["bb"], np.float32)
    for k in range(8):
        put(1 + k, Wb[k], Lb[k], bb[k])
    put(9, np.asarray(inputs["W2"], np.float32),
        np.asarray(inputs["L2"], np.float32),
        np.asarray(inputs["b2"], np.float32))

    common = {
        "wcat": wcat.astype(BF16), "lcq": lcq.astype(BF16),
        "bcol": bcol, "smat": s_host, "gidx": gidx_w,
    }
    return common, tile_block, KT


def kernel(**inputs):
    verts = np.asarray(inputs["verts_feats"], np.float32)   # [8, 6890, 128]
    Bsz = verts.shape[0]
    common, tile_block, KT = _host_arrays(inputs)

    key = (KT, tuple(tile_block))
    if key not in _CACHE:
        _CACHE.clear()
        nc = _build_program(tile_block, KT)
        _CACHE[key] = (nc, _make_dispatch(nc, Bsz))
    nc, dispatch = _CACHE[key]

    x0T = np.zeros((Bsz, 128, NP), np.float32)
    x0T[:, :, :N] = verts.transpose(0, 2, 1)
    x0T = x0T.astype(BF16)
    in_maps = [dict(common, x0T=x0T[b]) for b in range(Bsz)]
    if TRACE:
        res = bass_utils.run_bass_kernel_spmd(
            nc, in_maps, core_ids=list(range(Bsz)), trace=True)
        globals()['LAST_RESULTS'] = res
        results = res.results
    else:
        results = dispatch(in_maps)
    out = np.stack([results[b]["out"] for b in range(Bsz)], axis=0)
    return out.astype(np.float32)


if __name__ == "__main__":
    sys.path.insert(0, os.path.dirname(os.path.abspath(__file__)))
    import reference as R
    inputs = {k: np.asarray(v) for k, v in R.setup_inputs().items()}
    exp = np.asarray(R.reference(**R.setup_inputs()))
    got = kernel(**inputs)
    err = np.abs(got - exp).max() / np.abs(exp).max()
    print("Relative error:", err)


# revision 25
# speedup vs baseline: 9.1882x; 3.2782x over previous
"""Trainium2 Bass kernel for nn_MeshDeformation (GNN message passing).

Data-parallel over batch B=8 across 8 cores, one batch item per core.

Feature-major design: activations live in SBUF as xT [128 hid-part, 2 ktiles,
NP verts] so no transposes are ever needed:
  - phase M: mm = x@W vertex-major ([v,h] = xT_blk.T @ W) -> bf16 rows to a
    ping-pong HBM buffer. Software-pipelined: conv c+1's phase M is emitted
    inside conv c's gather/scatter loop (block i emitted once conv c finished
    block i, with a small lag so PE never stalls on the evacuation).
  - phase G: batched dma_gather (one SWDGE instruction per CH-tile chunk)
    pulls dst-sorted edge rows into SBUF edge-major; chunks double-buffered.
  - phase S: per 128-edge tile, 2 matmuls (hid halves): pacc_q += g_q.T @ S_t
    giving feature-major psum out [h-half, dst]; the x@L term accumulates into
    the same psum group (lhsT = L quarter, rhs = xT block); bias+ReLU fused
    into the scalar-engine evacuation (activation bias is per-partition =
    per-hid-feature here). Residual folds the *0.5 into the activation scale.
  - S matrices (val folded in) and gather indices stay resident in SBUF for
    all 10 convs.
  - final conv computes y=x@W2 into padded 512B rows, gathers y, single-half
    scatter, Tanh+bias evac, PE-transpose [3,128]->[128,3], x0.1 on evac.
"""
import sys, os
sys.path.insert(0, '/opt/trn_rl_repo')
import numpy as np
import ml_dtypes

import concourse.bass as bass
import concourse.bacc as bacc
import concourse.mybir as mybir
import concourse.tile as tile
from concourse import bass_utils
from concourse.library_config import mlp as _mlp_lib

N = 6890
NP = 6912          # padded vertices (54 * 128)
NB = NP // 128     # 54 dst/vertex blocks
HID = 256
FEAT = 128
NCONV = 10         # conv1, 8 hidden convs, final conv2
CH = 8             # gather tiles per chunk (1024 rows: HW dma_gather limit)
GB = 3             # phase-M blocks per staged HBM write
MLAG = 2           # blocks of lag for interleaved next-conv phase M
DEBUG_STAGE = 0
PIPELINE = True
BIAS_AP = True

BF16 = ml_dtypes.bfloat16


def _edge_tiles(src, dst, val):
    """dst-sorted, per-dst-block 128-padded edge tiling (vectorized).

    Returns (gidx [KT*128] int16 src ids (pad=0), S [KT,128,128] f32,
    tile_block [KT] int array).
    """
    blk = (dst >> 7).astype(np.int64)
    # one gathered row per unique (block, src); duplicate edges fold into S
    pair = blk * 8192 + src                      # unique (block, src) key
    uniq, inv = np.unique(pair, return_inverse=True)
    ublk = (uniq // 8192).astype(np.int64)
    usrc = (uniq % 8192).astype(np.int64)
    counts = np.bincount(ublk, minlength=NB)     # unique rows per block
    ntiles = (counts + 127) // 128
    KT = int(ntiles.sum())
    tile_block = np.repeat(np.arange(NB), ntiles)
    block_start_row = np.concatenate([[0], np.cumsum(counts)])[:-1]
    block_start_tile = np.concatenate([[0], np.cumsum(ntiles)])[:-1]
    nrow = len(uniq)
    within = np.arange(nrow) - block_start_row[ublk]   # row slot within block
    tile_of_row = block_start_tile[ublk] + (within >> 7)
    k_of_row = within & 127
    gidx = np.zeros(KT * 128, np.int16)
    gidx[tile_of_row * 128 + k_of_row] = usrc.astype(np.int16)
    S = np.zeros((KT, 128, 128), np.float32)
    np.add.at(S, (tile_of_row[inv], k_of_row[inv], dst & 127), val)
    return gidx, S, tile_block


def _src_of(c, A, Bb):
    if c == 0:
        return Bb          # x0T in ktile-0 region
    return A if c % 2 == 1 else Bb


def _dst_of(c):
    if c == 0:
        return 'A'
    if c == 9:
        return 'final'
    return 'B' if c % 2 == 1 else 'resid'


def _build_program(tile_block, KT):
    tile_block = list(tile_block)
    chunks = []          # (jlo, nt) with a short ramp so gather-0 lands fast
    j = 0
    for nt in (4, 8):
        if j < KT:
            nt = min(nt, KT - j)
            chunks.append((j, nt))
            j += nt
    while j < KT:
        nt = min(CH, KT - j)
        chunks.append((j, nt))
        j += nt
    nchunks = len(chunks)
    tile_chunk = np.zeros(KT, np.int64)
    for ci, (jlo, nt) in enumerate(chunks):
        tile_chunk[jlo:jlo + nt] = ci
    tiles_of = {}
    for j, b in enumerate(tile_block):
        tiles_of.setdefault(b, []).append(j)

    nc = bacc.Bacc("TRN2", target_bir_lowering=False, debug=False)
    bf = mybir.dt.bfloat16
    f32 = mybir.dt.float32

    x0_d = nc.dram_tensor("x0T", [128, NP], bf, kind="ExternalInput")
    wcat_d = nc.dram_tensor("wcat", [128, NCONV * 2 * HID], bf,
                            kind="ExternalInput")
    lcq_d = nc.dram_tensor("lcq", [128, NCONV * 4 * 128], bf,
                           kind="ExternalInput")
    bcol_d = nc.dram_tensor("bcol", [128, NCONV * 4], f32,
                            kind="ExternalInput")
    s_d = nc.dram_tensor("smat", [128, KT * 128], bf, kind="ExternalInput")
    gidx_d = nc.dram_tensor("gidx", [128, KT * 8], mybir.dt.int16,
                            kind="ExternalInput")
    out_d = nc.dram_tensor("out", [N, 3], f32, kind="ExternalOutput")
    if DEBUG_STAGE:
        dbg_d = nc.dram_tensor("dbg", [128, 2 * NP], bf, kind="ExternalOutput")

    from concourse.masks import make_identity

    with tile.TileContext(nc) as tc:
        with (
            tc.tile_pool(name="dram", bufs=1, space="DRAM") as dram,
            tc.tile_pool(name="res", bufs=1) as res,
            tc.tile_pool(name="gpool", bufs=4) as gpool,
            tc.tile_pool(name="stg", bufs=3) as stg,
            tc.tile_pool(name="rstg", bufs=4) as rstg,
            tc.tile_pool(name="pmp", bufs=3, space="PSUM") as pmp,
            tc.tile_pool(name="acc", bufs=3, space="PSUM") as acc,
            tc.tile_pool(name="ptp", bufs=2, space="PSUM") as ptp,
        ):
            mm_a = dram.tile([NP, HID], bf, tag="mm0")
            mm_b = dram.tile([NP, HID], bf, tag="mm1")
            mm_pp = [mm_a, mm_b]

            S_all = res.tile([128, KT * 128], bf, tag="S")
            gix = res.tile([128, KT * 8], mybir.dt.int16, tag="gix")
            A = res.tile([128, 2 * NP], bf, tag="A")
            Bb = res.tile([128, 2 * NP], bf, tag="B")
            wc = res.tile([128, NCONV * 2 * HID], bf, tag="wc")
            lcq = res.tile([128, NCONV * 4 * 128], bf, tag="lcq")
            bcol = res.tile([128, NCONV * 4], f32, tag="bcol")
            id32 = res.tile([128, 128], f32, tag="id32")
            idbf = res.tile([128, 128], bf, tag="idbf")
            ostage = res.tile([128, NB * 3], f32, tag="ostage")
            snt = res.tile([128, 8], f32, tag="snt")
            sdm = res.tile([128, 8], f32, tag="sdm")

            nc.gpsimd.load_library(_mlp_lib)
            nc.sync.dma_start(out=wc[:], in_=wcat_d[:])
            nc.sync.dma_start(out=Bb[:, 0:NP], in_=x0_d[:])
            nc.sync.dma_start(out=lcq[:], in_=lcq_d[:])
            nc.sync.dma_start(out=bcol[:], in_=bcol_d[:])
            nc.sync.dma_start(out=gix[:], in_=gidx_d[:])
            make_identity(nc, id32[:])
            nc.vector.tensor_copy(out=idbf[:], in_=id32[:])

            nregs = {}
            for (_, nt) in chunks:
                if nt not in nregs:
                    nregs[nt] = nc.gpsimd.to_reg(nt * 128)

            def xs(src_tile, h, i):
                return src_tile[:, h * NP + i * 128: h * NP + (i + 1) * 128]

            # ---- phase M emitter (per-block, staged writes of GB blocks) ----
            def m_state(c, src_tile, mm_d):
                return dict(c=c, src=src_tile, mm=mm_d, ms=None, base=0, cnt=0)

            def emit_m_block(st, i):
                c, src_tile, mm_d = st['c'], st['src'], st['mm']
                fin = 1 if c == 0 else 2
                pm = pmp.tile([128, HID], f32, tag="pm", name="pm")
                for h in range(fin):
                    nc.tensor.matmul(
                        out=pm[:], lhsT=xs(src_tile, h, i),
                        rhs=wc[:, (2 * c + h) * HID:(2 * c + h + 1) * HID],
                        start=(h == 0), stop=(h == fin - 1))
                if st['cnt'] == 0:
                    st['ms'] = stg.tile([128, GB * HID], bf, tag="ms",
                                        name="ms")
                    st['base'] = i
                sl = st['cnt']
                if i % 2 == 0:
                    nc.vector.tensor_copy(
                        out=st['ms'][:, sl * HID:(sl + 1) * HID], in_=pm[:])
                else:
                    nc.scalar.copy(
                        out=st['ms'][:, sl * HID:(sl + 1) * HID], in_=pm[:])
                st['cnt'] += 1
                if st['cnt'] == GB or i == NB - 1:
                    lo, nb = st['base'], st['cnt']
                    nc.sync.dma_start(
                        out=mm_d[lo * 128:(lo + nb) * 128, :].rearrange(
                            "(b p) h -> p b h", p=128),
                        in_=st['ms'][:, :nb * HID].rearrange(
                            "p (b h) -> p b h", h=HID))
                    st['cnt'] = 0
                if i == NB - 1:
                    nc.sync.dma_start(out=snt[:], in_=bcol_d[:, 0:8])

            # ---- gather + scatter phase for one conv ----
            def conv_gs(c, src_tile, dst_mode, mm_d, nxt):
                fin = 1 if c == 0 else 2
                final = dst_mode == 'final'
                nq = 1 if final else 2

                gtiles = {}
                issued = [0]
                fpend = []
                # final conv: y lives in cols 0:128 of the 256-wide mm rows;
                # gather only 256B per row
                gel = 128 if final else HID

                def need_chunk(k):
                    while issued[0] <= min(k + 1, nchunks - 1):
                        ci = issued[0]
                        jlo, nt = chunks[ci]
                        if c == 0:
                            # stream the resident S in step with conv0's use
                            nc.sync.dma_start(
                                out=S_all[:, jlo * 128:(jlo + nt) * 128],
                                in_=s_d[:, jlo * 128:(jlo + nt) * 128])
                        gt = gpool.tile([128, CH * HID], bf, tag="g", name="g")
                        nc.gpsimd.dma_gather(
                            gt[:, :nt * gel].rearrange("p (t e) -> p t e",
                                                       e=gel),
                            mm_d[:, 0:gel], gix[:, jlo * 8:(jlo + nt) * 8],
                            nt * 128, nregs[nt], gel,
                            elem_step=HID)
                        gtiles[ci] = gt
                        issued[0] += 1
                    return gtiles[k]

                def lterm(i, pacc2, has_tiles):
                    for q in range(nq):
                        for h in range(fin):
                            nc.tensor.matmul(
                                out=pacc2[q][:],
                                lhsT=lcq[:, (c * 4 + h * 2 + q) * 128:
                                         (c * 4 + h * 2 + q + 1) * 128],
                                rhs=xs(src_tile, h, i),
                                start=(h == 0),
                                stop=(h == fin - 1) and not has_tiles)

                def finish(i, pacc2, started):
                    for q in range(nq):
                        pq = pacc2[q][:]
                        if dst_mode in ('A', 'B'):
                            dbuf = A if dst_mode == 'A' else Bb
                            nc.scalar.activation(
                                out=dbuf[:, q * NP + i * 128:
                                         q * NP + (i + 1) * 128],
                                in_=pq,
                                func=mybir.ActivationFunctionType.Relu,
                                bias=(bcol[:, c * 4 + q: c * 4 + q + 1]
                                      if BIAS_AP else 0.0))
                        elif dst_mode == 'resid':
                            # A = 0.5*A + relu(0.5*pacc + 0.5*b)
                            asl = A[:, q * NP + i * 128: q * NP + (i + 1) * 128]
                            t = rstg.tile([128, 128], bf, tag="rt", name="rt")
                            nc.scalar.activation(
                                out=t[:], in_=pq,
                                func=mybir.ActivationFunctionType.Relu,
                                scale=0.5,
                                bias=bcol[:, c * 4 + 2 + q: c * 4 + 2 + q + 1])
                            nc.vector.tensor_scalar_mul(asl, asl, 0.5)
                            nc.vector.tensor_tensor(
                                out=asl, in0=asl, in1=t[:],
                                op=mybir.AluOpType.add)
                        else:  # final
                            t = rstg.tile([128, 128], bf, tag="tt", name="tt")
                            nc.scalar.activation(
                                out=t[0:3, :], in_=pacc2[0][0:3, :],
                                func=mybir.ActivationFunctionType.Tanh,
                                bias=bcol[0:3, c * 4: c * 4 + 1])
                            fpend.append((i, t))

                def flush_final(i):
                    while fpend and fpend[0][0] <= i:
                        fi, t = fpend.pop(0)
                        pt = ptp.tile([128, 128], bf, tag="pt", name="pt")
                        nc.tensor.transpose(
                            out=pt[:, 0:3], in_=t[0:3, :],
                            identity=idbf[0:3, 0:3])
                        nc.scalar.mul(
                            out=ostage[:, fi * 3:(fi + 1) * 3],
                            in_=pt[:, 0:3], mul=0.1)

                for i in range(NB):
                    pacc2 = [acc.tile([128, 128], f32, tag="pacc", name="pacc")
                             for _ in range(nq)]
                    tj = tiles_of.get(i, [])
                    lterm(i, pacc2, bool(tj))
                    for j in tj:
                        k = int(tile_chunk[j])
                        gt = need_chunk(k)
                        jj = j - chunks[k][0]
                        last = j == tj[-1]
                        for q in range(nq):
                            nc.tensor.matmul(
                                out=pacc2[q][:],
                                lhsT=gt[:, jj * gel + q * 128:
                                        jj * gel + (q + 1) * 128],
                                rhs=S_all[:, j * 128:(j + 1) * 128],
                                start=False, stop=last)
                    finish(i, pacc2, [True, True])
                    if final:
                        flush_final(i - MLAG)
                    if nxt is not None and i >= MLAG:
                        emit_m_block(nxt, i - MLAG)
                if nxt is not None:
                    for i in range(NB - MLAG, NB):
                        emit_m_block(nxt, i)
                if final:
                    flush_final(NB)

                if final:
                    nfull = N // 128  # 53 full blocks
                    nc.gpsimd.dma_start(
                        out=out_d[0:nfull * 128, :].rearrange(
                            "(i p) c -> p i c", p=128),
                        in_=ostage[:, 0:nfull * 3].rearrange(
                            "p (i c) -> p i c", c=3))
                    rem = N - nfull * 128
                    nc.gpsimd.dma_start(
                        out=out_d[nfull * 128:N, :],
                        in_=ostage[0:rem, nfull * 3:(nfull + 1) * 3])

            # ---- network ----
            ncv = 10 if DEBUG_STAGE == 0 else {1: 1, 2: 2, 3: 3, 9: 9}[DEBUG_STAGE]
            states = [None] * (NCONV + 1)
            states[0] = m_state(0, _src_of(0, A, Bb), mm_pp[0])
            for i in range(NB):
                emit_m_block(states[0], i)
            for c in range(ncv):
                # Pool-issued SBUF->SBUF DMA reading the sentinel: Q7 waits
                # for the sentinel HWDGE write (FIFO after all mm writes), so
                # every later gather sees a fully-written mm buffer.
                nc.gpsimd.dma_start(out=sdm[:], in_=snt[:])
                nxt = None
                if c + 1 < ncv:
                    states[c + 1] = m_state(c + 1, _src_of(c + 1, A, Bb),
                                            mm_pp[(c + 1) % 2])
                    if PIPELINE:
                        nxt = states[c + 1]
                conv_gs(c, _src_of(c, A, Bb), _dst_of(c), mm_pp[c % 2], nxt)
                if not PIPELINE and c + 1 < ncv:
                    for i in range(NB):
                        emit_m_block(states[c + 1], i)
            if DEBUG_STAGE:
                dsrc = A if DEBUG_STAGE in (1, 3, 9) else Bb
                nc.sync.dma_start(out=dbg_d[:], in_=dsrc[:])

    nc.finalize()
    return nc


_CACHE = {}
TRACE = False
LAST_RESULTS = None


def _make_dispatch(nc, n_cores):
    """Build a cached PJRT dispatcher (mirrors bass2jax.run_bass_via_pjrt but
    traces/compiles the jitted callable once instead of per call)."""
    import jax
    from jax.sharding import Mesh, PartitionSpec
    from jax.experimental.shard_map import shard_map
    from concourse import bass2jax
    import concourse.mybir as mb

    bass2jax.install_neuronx_cc_hook()
    partition_name = (nc.partition_id_tensor.name
                      if nc.partition_id_tensor else None)
    in_names, out_names, out_avals, zero_outs = [], [], [], []
    for alloc in nc.m.functions[0].allocations:
        if not isinstance(alloc, mb.MemoryLocationSet):
            continue
        name = alloc.memorylocations[0].name
        if alloc.kind == "ExternalInput":
            if name != partition_name:
                in_names.append(name)
        elif alloc.kind == "ExternalOutput":
            shape = tuple(alloc.tensor_shape)
            dtype = mb.dt.np(alloc.dtype)
            out_names.append(name)
            out_avals.append(jax.core.ShapedArray(shape, dtype))
            zero_outs.append(np.zeros(shape, dtype))
    n_params = len(in_names)
    n_outs = len(out_avals)
    all_in_names = list(in_names) + list(out_names)
    if partition_name is not None:
        all_in_names.append(partition_name)
    donate = tuple(range(n_params, n_params + n_outs))

    def _body(*args):
        operands = list(args)
        if partition_name is not None:
            operands.append(bass2jax.partition_id_tensor())
        outs = bass2jax._bass_exec_p.bind(
            *operands,
            out_avals=tuple(out_avals),
            in_names=tuple(all_in_names),
            out_names=tuple(out_names),
            lowering_input_output_aliases=(),
            sim_require_finite=True,
            sim_require_nnan=True,
            nc=nc,
        )
        return tuple(outs)

    devices = jax.devices()[:n_cores]
    mesh = Mesh(np.asarray(devices), ("core",))
    in_specs = (PartitionSpec("core"),) * (n_params + n_outs)
    out_specs = (PartitionSpec("core"),) * n_outs
    sharded = jax.jit(
        shard_map(_body, mesh=mesh, in_specs=in_specs, out_specs=out_specs,
                  check_rep=False),
        donate_argnums=donate, keep_unused=True)

    from jax.sharding import NamedSharding
    shard = NamedSharding(mesh, PartitionSpec("core"))
    dev_cache = {}   # name -> (digest, device array); replicated inputs only

    def run(in_maps):
        import hashlib
        concat_in = []
        for i, name in enumerate(in_names):
            if name == "x0T":
                concat_in.append(np.concatenate(
                    [np.asarray(m[name]) for m in in_maps], axis=0))
                continue
            # identical across cores: keep device-resident, keyed by content
            arr = np.asarray(in_maps[0][name])
            dig = hashlib.md5(arr.tobytes()).hexdigest()
            hit = dev_cache.get(name)
            if hit is None or hit[0] != dig:
                ga = jax.device_put(
                    np.concatenate([arr] * n_cores, axis=0), shard)
                dev_cache[name] = (dig, ga)
            concat_in.append(dev_cache[name][1])
        concat_zeros = [
            np.zeros((n_cores * z.shape[0], *z.shape[1:]), z.dtype)
            for z in zero_outs]
        out_arrs = sharded(*concat_in, *concat_zeros)
        return [
            {name: np.asarray(out_arrs[i]).reshape(
                n_cores, *out_avals[i].shape)[c]
             for i, name in enumerate(out_names)}
            for c in range(n_cores)]

    return run


def _host_arrays(inputs):
    src = np.asarray(inputs["edge_src"]).astype(np.int64)
    dst = np.asarray(inputs["edge_dst"]).astype(np.int64)
    val = np.asarray(inputs["edge_val"], np.float32)

    gidx, S, tile_block = _edge_tiles(src, dst, val)
    KT = len(tile_block)
    s_host = np.ascontiguousarray(
        S.transpose(1, 0, 2).reshape(128, KT * 128)).astype(BF16)
    gidx_w = np.ascontiguousarray(
        np.tile(gidx.reshape(KT * 8, 16).T, (8, 1)))

    wcat = np.zeros((128, NCONV * 2 * HID), np.float32)
    lcq = np.zeros((128, NCONV * 4 * 128), np.float32)
    # bcol layout per conv c: col c*4+q = b[q-half]; col c*4+2+q = 0.5*b
    bcol = np.zeros((128, NCONV * 4), np.float32)

    def put(c, W, L, b):
        nh = W.shape[0] // 128
        no = W.shape[1]
        for h in range(nh):
            wcat[:, (2 * c + h) * HID:(2 * c + h) * HID + no] = \
                W[h * 128:(h + 1) * 128]
            for q in range(2):
                qs = slice(q * 128, min((q + 1) * 128, no))
                ncol = qs.stop - qs.start
                if ncol <= 0:
                    continue
                lcq[:, (c * 4 + h * 2 + q) * 128:
                    (c * 4 + h * 2 + q) * 128 + ncol] = \
                    L[h * 128:(h + 1) * 128, qs]
        for q in range(2):
            qs = slice(q * 128, min((q + 1) * 128, len(b)))
            ncol = qs.stop - qs.start
            if ncol <= 0:
                continue
            bcol[0:ncol, c * 4 + q] = b[qs]
            bcol[0:ncol, c * 4 + 2 + q] = 0.5 * b[qs]

    put(0, np.asarray(inputs["W1"], np.float32),
        np.asarray(inputs["L1"], np.float32),
        np.asarray(inputs["b1"], np.float32))
    Wb = np.asarray(inputs["Wb"], np.float32)
    Lb = np.asarray(inputs["Lb"], np.float32)
    bb = np.asarray(inputs["bb"], np.float32)
    for k in range(8):
        put(1 + k, Wb[k], Lb[k], bb[k])
    put(9, np.asarray(inputs["W2"], np.float32),
        np.asarray(inputs["L2"], np.float32),
        np.asarray(inputs["b2"], np.float32))

    common = {
        "wcat": wcat.astype(BF16), "lcq": lcq.astype(BF16),
        "bcol": bcol, "smat": s_host, "gidx": gidx_w,
    }
    return common, tile_block, KT


def kernel(**inputs):
    verts = np.asarray(inputs["verts_feats"], np.float32)   # [8, 6890, 128]
    Bsz = verts.shape[0]
    common, tile_block, KT = _host_arrays(inputs)

    key = (KT, tuple(tile_block))
    if key not in _CACHE:
        _CACHE.clear()
        nc = _build_program(tile_block, KT)
        _CACHE[key] = (nc, _make_dispatch(nc, Bsz))
    nc, dispatch = _CACHE[key]

    x0T = np.zeros((Bsz, 128, NP), np.float32)
    x0T[:, :, :N] = verts.transpose(0, 2, 1)
    x0T = x0T.astype(BF16)
    in_maps = [dict(common, x0T=x0T[b]) for b in range(Bsz)]
    if TRACE:
        res = bass_utils.run_bass_kernel_spmd(
            nc, in_maps, core_ids=list(range(Bsz)), trace=True)
        globals()['LAST_RESULTS'] = res
        results = res.results
    else:
        results = dispatch(in_maps)
    out = np.stack([results[b]["out"] for b in range(Bsz)], axis=0)
    return out.astype(np.float32)


if __name__ == "__main__":
    sys.path.insert(0, os.path.dirname(os.path.abspath(__file__)))
    import reference as R
    inputs = {k: np.asarray(v) for k, v in R.setup_inputs().items()}
    exp = np.asarray(R.reference(**R.setup_inputs()))
    got = kernel(**inputs)
    err = np.abs(got - exp).max() / np.abs(exp).max()
    print("Relative error:", err)
